# revision 7
# baseline (speedup 1.0000x reference)
"""Block-circulant matmul kernel for Trainium2 (8 NeuronCores, data-parallel).

Computes out = (x * D) @ M + bias where M is the 4096x4096 block-circulant
matrix built from W[32, 32, 128] (block (i,j) is C_ij[s,t] = W[i,j,(s-t)%128]).

Sharding: batch (4096) split 8 ways -> 512 rows per core; weights replicated.

Implementation "fft2" (default): 3-stage frequency-domain factorization in
bf16 (fp32 PSUM accumulation). Per core:
 - host folds the Bernoulli diagonal into x and ships xT as bf16;
 - stage A: DFT-as-matmul, ONE shared stationary Csig for all 32 blocks;
 - DVE 32x32 stream-transpose; per-slot block-diag matmul (32 stationaries);
 - DVE transpose; iDFT-as-matmul with shared Esig stationary + bias in the
   PSUM-evacuation copies; bf16 output DMA, host upcasts.
The sigma frequency packing puts the 4 real components of a frequency
pair-slot c at spectrum positions {c, 32+c, 64+c, 96+c} so the
quadrant-local DVE transpose lands rows exactly where the next stage's
matmul needs them.

Legacy impls "fft" (fp32 3-stage) and "dense" kept for fallback via BC_IMPL.
"""

import os
import numpy as np

import concourse.bass as bass
import concourse.mybir as mybir
from concourse import bacc
from concourse.tile import TileContext
from concourse.bass_utils import run_bass_kernel_spmd
import concourse.bass_utils as _bu

# Let walrus overlap LDWEIGHTS with in-flight matmuls.
LDWOPT = os.environ.get("BC_LDWOPT", "0") == "1"
if not getattr(_bu, "_bc_ldwopt_patched", False):
    _bu._bc_ldwopt_patched = True
    _orig_bvo = _bu.bir_verify_and_optimise

    def _bvo_ldwopt(*a, **k):
        orig_rc = _bu.run_command

        def rc(argv, **kw):
            if LDWOPT:
                argv = [s.replace("--enable-ldw-opt=false",
                                  "--enable-ldw-opt=true") for s in argv]
            return orig_rc(argv, **kw)

        _bu.run_command = rc
        try:
            return _orig_bvo(*a, **k)
        finally:
            _bu.run_command = orig_rc

    _bu.bir_verify_and_optimise = _bvo_ldwopt

# Problem constants (hardcoded per harness contract).
BATCH = 4096
D_IN = 4096
D_OUT = 4096
BS = 128          # circulant block size
KI = 32           # input blocks
KO = 32           # output blocks
NCORES = 8
BC = BATCH // NCORES      # 512 batch rows per core
NSPLIT = 2                # batch halves per core (pipeline + PSUM sizing)
BH = BC // NSPLIT

IMPL = os.environ.get("BC_IMPL", "fft3")
MM_DTYPE = os.environ.get("BC_DTYPE", "fp32")   # legacy fft/dense impls only
# engine pattern for each psum-evacuation copy family, cycled per call:
# 'a' = ACT (scalar), 'd' = DVE (vector).  PSUM is not reachable from Pool.
ENG_A = os.environ.get("BC_ENG_A", "a")
ENG_B = os.environ.get("BC_ENG_B", "a")
ENG_C = os.environ.get("BC_ENG_C", "ad")
# bias mechanism: "pe" = 1-partition ones-matmul into PSUM, "evac" = per-i
# Identity+bias during C evacuation
CBIAS = os.environ.get("BC_CBIAS", "pe")
# intermediate dtype: bf16 (fast, known-good) or f32r (fp32 bits, accuracy fallback)
IDT_NAME = os.environ.get("BC_IDT", "bf16")

_NC_CACHE = {}
_PACK_CACHE = {}


def _dt_of(name):
    return {
        "fp32": mybir.dt.float32,
        "f32r": mybir.dt.float32r,
        "bf16": mybir.dt.bfloat16,
    }[name]


def _bf16():
    import ml_dtypes
    return ml_dtypes.bfloat16


# ---------------------------------------------------------------- sigma pack
def _sigma_components():
    """slot c, quadrant Q -> ("re"|"im", f). Pairs (2c+1, 2c+2) for c<31,
    slot 31 holds (63 complex, 0 real, 64 real)."""
    comp = {}
    for c in range(32):
        fa = 2 * c + 1 if c < 31 else 63
        comp[(0, c)] = ("re", fa)
        comp[(1, c)] = ("im", fa)
        if c < 31:
            comp[(2, c)] = ("re", 2 * c + 2)
            comp[(3, c)] = ("im", 2 * c + 2)
        else:
            comp[(2, c)] = ("re", 0)
            comp[(3, c)] = ("re", 64)
    return comp


def _pack_const():
    """Input-independent factor matrices Csig [s, m] and Esig [m, t]."""
    if "const" in _PACK_CACHE:
        return _PACK_CACHE["const"]
    comp = _sigma_components()
    s = np.arange(BS)
    Csig = np.zeros((BS, 128), dtype=np.float64)
    Esig = np.zeros((128, BS), dtype=np.float64)
    for (Q, c), (typ, f) in comp.items():
        m = 32 * Q + c
        ang = 2 * np.pi * f * s / BS
        a = (1.0 if f in (0, 64) else 2.0) / BS
        if typ == "re":
            Csig[:, m] = np.cos(ang)
            Esig[m, :] = a * np.cos(ang)
        else:
            Csig[:, m] = -np.sin(ang)
            Esig[m, :] = -a * np.sin(ang)
    out = (Csig.astype(np.float32), np.ascontiguousarray(Esig.astype(np.float32)))
    _PACK_CACHE["const"] = out
    return out


def _pack_wb(W):
    """Frequency-domain block-diagonal weights WBt [row=(Qr,j), slot, col=(Qc,i)]."""
    comp = _sigma_components()
    Wf = np.fft.fft(W.astype(np.float64), axis=-1)
    Wfr, Wfi = Wf.real, Wf.imag
    WB = np.zeros((32, 128, 128), dtype=np.float64)
    for c in range(32):
        for (qre, qim) in ((0, 1), (2, 3)):
            typ_im = comp[(qim, c)][0]
            f = comp[(qre, c)][1]
            if typ_im == "im":
                wr = Wfr[:, :, f].T  # [j, i]
                wi = Wfi[:, :, f].T
                WB[c, qre*32:(qre+1)*32, qre*32:(qre+1)*32] = wr
                WB[c, qim*32:(qim+1)*32, qre*32:(qre+1)*32] = wi
                WB[c, qre*32:(qre+1)*32, qim*32:(qim+1)*32] = -wi
                WB[c, qim*32:(qim+1)*32, qim*32:(qim+1)*32] = wr
            else:
                f2 = comp[(qim, c)][1]
                WB[c, qre*32:(qre+1)*32, qre*32:(qre+1)*32] = Wfr[:, :, f].T
                WB[c, qim*32:(qim+1)*32, qim*32:(qim+1)*32] = Wfr[:, :, f2].T
    return np.ascontiguousarray(
        WB.transpose(1, 0, 2).astype(np.float32)  # [row, slot, col]
    )


# --------------------------------------------------------------- fft3 build
def _build_fft3():
    key = ("fft3", ENG_A, ENG_B, ENG_C, CBIAS, IDT_NAME)
    if key in _NC_CACHE:
        return _NC_CACHE[key]
    DT = mybir.dt.bfloat16            # x / csig / output dtype
    IDT = _dt_of(IDT_NAME)            # intermediate + B/C weights dtype
    f32 = mybir.dt.float32

    nc = bacc.Bacc(None, target_bir_lowering=False, debug=False)

    xT = nc.dram_tensor("xT", [BS, KI, BC], DT, kind="ExternalInput")
    Csig_d = nc.dram_tensor("Csig", [BS, 128], DT, kind="ExternalInput")
    WBt_d = nc.dram_tensor("WBt", [128, 32, 128], IDT, kind="ExternalInput")
    Esig_d = nc.dram_tensor("Esig", [128, BS], IDT, kind="ExternalInput")
    bT_d = nc.dram_tensor("bT", [BS, KO], f32, kind="ExternalInput")
    bRow_d = nc.dram_tensor("bRow", [1, KO, BS], IDT, kind="ExternalInput")
    outT = nc.dram_tensor("outT", [KO, BS, BC], DT, kind="ExternalOutput")
    if LDWOPT:
        nc.dram_tensor("ldwopt_tag", [1, 1], f32, kind="ExternalInput")

    def eng_iter(pattern):
        n = [0]

        def next_eng():
            e = pattern[n[0] % len(pattern)]
            n[0] += 1
            return e
        return next_eng

    ea, eb, ec = eng_iter(ENG_A), eng_iter(ENG_B), eng_iter(ENG_C)

    def evac(eng, out, in_, bias=None):
        """PSUM -> SBUF evacuation copy on ACT ('a') or DVE ('d')."""
        if bias is not None:
            if eng == "a":
                nc.scalar.activation(
                    out=out, in_=in_,
                    func=mybir.ActivationFunctionType.Identity, bias=bias,
                )
            else:
                nc.vector.tensor_scalar_add(out=out, in0=in_, scalar1=bias)
        else:
            if eng == "a":
                nc.scalar.activation(
                    out=out, in_=in_, func=mybir.ActivationFunctionType.Copy
                )
            else:
                nc.vector.tensor_copy(out=out, in_=in_)

    with TileContext(nc) as tc:
        with tc.tile_pool(name="consts", bufs=1) as cpool, \
             tc.tile_pool(name="stage", bufs=8) as spool, \
             tc.tile_pool(name="big1", bufs=2) as big1, \
             tc.tile_pool(name="big2", bufs=2) as big2, \
             tc.tile_pool(name="o", bufs=4) as opool, \
             tc.tile_pool(name="psAC", bufs=2, space="PSUM") as psAC, \
             tc.tile_pool(name="psB", bufs=2, space="PSUM") as psB:

            csig = cpool.tile([BS, 128], DT)
            esig = cpool.tile([128, BS], IDT)
            wb = cpool.tile([128, 32, 128], IDT)
            bt_t = cpool.tile([BS, KO], f32)
            brow = cpool.tile([1, KO, BS], IDT)
            ones = cpool.tile([1, BH], IDT)
            nc.sync.dma_start(out=csig, in_=Csig_d[:, :])
            nc.sync.dma_start(out=esig, in_=Esig_d[:, :])
            nc.sync.dma_start(out=wb, in_=WBt_d[:, :, :])
            nc.sync.dma_start(out=bt_t, in_=bT_d[:, :])
            nc.sync.dma_start(out=brow, in_=bRow_d[:, :, :])
            nc.vector.memset(ones, 1.0)

            # xf/yz: b-major (transpose reads contiguous); z/yw: slot-major
            # (matmul moving slices contiguous)
            xf = [big1.tile([128, BH, KI], IDT, tag="big1", name=f"xf{h}")
                  for h in range(NSPLIT)]
            z = [None] * NSPLIT
            yz = [None] * NSPLIT
            yw = [None] * NSPLIT

            def stage_a(h):
                # DFT: XF[m, b, j] = sum_s Csig[s, m] * xd[s, j, b]
                for q in range(KI // 4):
                    ps = psAC.tile([128, 4, BH], f32, tag="psAC",
                                   name=f"psa{q}_{h}")
                    for d in range(4):
                        j = 4 * q + d
                        st = spool.tile([BS, BH], DT, tag="stage")
                        nc.sync.dma_start(
                            out=st, in_=xT[:, j, h * BH:(h + 1) * BH]
                        )
                        nc.tensor.matmul(
                            ps[:, d, :], csig, st, start=True, stop=True
                        )
                    evac(ea(), xf[h][:, :, 4 * q:4 * q + 4],
                         ps.transpose([0, 2, 1]))

            def t1(h):
                # Z[(Q,j), c, b] = XF[(Q,c), b, j]  (in contiguous, out strided)
                z[h] = big2.tile([128, 32, BH], IDT, tag="big2", name=f"z{h}")
                nc.vector.transpose(out=z[h].transpose([0, 2, 1]), in_=xf[h])

            def stage_b(h):
                # per-slot block-diagonal frequency matmul
                yz[h] = big1.tile([128, BH, 32], IDT, tag="big1", name=f"yz{h}")
                for q in range(8):
                    ps = psB.tile([128, 4, BH], f32, tag="psB",
                                  name=f"psb{q}_{h}")
                    for d in range(4):
                        c = 4 * q + d
                        nc.tensor.matmul(
                            ps[:, d, :], wb[:, c, :], z[h][:, c, :],
                            start=True, stop=True,
                        )
                    evac(eb(), yz[h][:, :, 4 * q:4 * q + 4],
                         ps.transpose([0, 2, 1]))

            def t2(h):
                # YW[(Q,c), i, b] = YZ[(Q,i), b, c]  (in contiguous, out strided)
                yw[h] = big2.tile([128, 32, BH], IDT, tag="big2", name=f"yw{h}")
                nc.vector.transpose(out=yw[h].transpose([0, 2, 1]), in_=yz[h])

            def stage_c(h):
                # iDFT + bias; esig shared stationary, 4 output blocks per psum
                for q in range(KO // 4):
                    ps = psAC.tile([128, 4, BH], f32, tag="psAC",
                                   name=f"psc{q}_{h}")
                    for p in range(2):
                        sl = slice(4 * q + 2 * p, 4 * q + 2 * p + 2)
                        nc.tensor.matmul(
                            ps[:, 2 * p:2 * p + 2, :], esig, yw[h][:, sl, :],
                            start=True, stop=(CBIAS != "pe"),
                        )
                    if CBIAS == "pe":
                        for d in range(4):
                            i = 4 * q + d
                            nc.tensor.matmul(
                                ps[:, d, :], brow[:, i, :], ones,
                                start=False, stop=(d % 2 == 1),
                                skip_group_check=True,
                            )
                        ob = opool.tile([128, 4, BH], DT, tag="o")
                        evac(ec(), ob, ps)
                        for d in range(4):
                            i = 4 * q + d
                            nc.sync.dma_start(
                                out=outT[i, :, h * BH:(h + 1) * BH],
                                in_=ob[:, d, :],
                            )
                    else:
                        ob = opool.tile([128, 4, BH], DT, tag="o")
                        for d in range(4):
                            i = 4 * q + d
                            evac(ec(), ob[:, d, :], ps[:, d, :],
                                 bias=bt_t[:, i:i + 1])
                            nc.sync.dma_start(
                                out=outT[i, :, h * BH:(h + 1) * BH],
                                in_=ob[:, d, :],
                            )

            # dataflow-ordered emission for pipelined queues
            stage_a(0)
            t1(0)
            stage_a(1)
            stage_b(0)
            t1(1)
            t2(0)
            stage_b(1)
            stage_c(0)
            t2(1)
            stage_c(1)

    nc.compile()
    _NC_CACHE[key] = nc
    return nc


def _prep_fft3(x, W, D, bias):
    bf = _bf16()
    Csig, Esig = _pack_const()
    WBt = _pack_wb(W)
    bT = np.ascontiguousarray(bias.reshape(KO, BS).T).astype(np.float32)
    bRow = np.ascontiguousarray(bias.reshape(1, KO, BS))
    Csig16 = Csig.astype(bf)
    wdt = bf if IDT_NAME == "bf16" else np.float32
    Esig_w = np.ascontiguousarray(Esig).astype(wdt)
    WBt_w = WBt.astype(wdt)
    bRow_w = bRow.astype(wdt)
    xd = (x * D[None, :]).astype(np.float32)
    in_maps = []
    for c in range(NCORES):
        xs = xd[c * BC:(c + 1) * BC, :]
        xTc = np.ascontiguousarray(
            xs.reshape(BC, KI, BS).transpose(2, 1, 0)
        ).astype(bf)
        im = {"xT": xTc, "Csig": Csig16, "WBt": WBt_w, "Esig": Esig_w,
              "bT": bT, "bRow": bRow_w}
        if LDWOPT:
            im["ldwopt_tag"] = np.zeros((1, 1), dtype=np.float32)
        in_maps.append(im)
    return in_maps


# ---------------------------------------------------------------- fft build
def _build_fft(mm_dtype):
    key = ("fft", mm_dtype)
    if key in _NC_CACHE:
        return _NC_CACHE[key]
    DT = _dt_of(mm_dtype)
    f32 = mybir.dt.float32
    need_round = DT == mybir.dt.float32r
    nsplit = 2
    bh = BC // nsplit

    nc = bacc.Bacc(None, target_bir_lowering=False, debug=False)

    xT = nc.dram_tensor("xT", [BS, KI, BC], DT, kind="ExternalInput")
    Csig_d = nc.dram_tensor("Csig", [BS, 128], f32, kind="ExternalInput")
    WBt_d = nc.dram_tensor("WBt", [128, 32, 128], DT, kind="ExternalInput")
    Esig_d = nc.dram_tensor("Esig", [128, BS], DT, kind="ExternalInput")
    Dt_d = nc.dram_tensor("Dt", [BS, KI], f32, kind="ExternalInput")
    bT_d = nc.dram_tensor("bT", [BS, KO], f32, kind="ExternalInput")
    outT = nc.dram_tensor("outT", [KO, BS, BC], f32, kind="ExternalOutput")
    if LDWOPT:
        nc.dram_tensor("ldwopt_tag", [1, 1], f32, kind="ExternalInput")

    def do_copy(k, out, in_, act_frac=2):
        if k % act_frac == 0:
            nc.vector.tensor_copy(out=out, in_=in_)
        else:
            nc.scalar.activation(
                out=out, in_=in_, func=mybir.ActivationFunctionType.Copy
            )

    with TileContext(nc) as tc:
        with tc.tile_pool(name="consts", bufs=1) as cpool, \
             tc.tile_pool(name="stage", bufs=6) as spool, \
             tc.tile_pool(name="big1", bufs=2) as big1, \
             tc.tile_pool(name="big2", bufs=2) as big2, \
             tc.tile_pool(name="big3", bufs=1) as big3, \
             tc.tile_pool(name="o", bufs=6) as opool, \
             tc.tile_pool(name="psAll", bufs=8, space="PSUM") as psAll:

            psA = psB = psC = psAll
            csig = cpool.tile([BS, 128], f32)
            esig = cpool.tile([128, BS], DT)
            dt_t = cpool.tile([BS, KI], f32)
            bt_t = cpool.tile([BS, KO], f32)
            wb = cpool.tile([128, 32, 128], DT)
            if need_round:
                cd = big3.tile([BS, KI, 128], DT, tag="big3", name="cd")
            else:
                cd = cpool.tile([BS, KI, 128], DT)
            nc.sync.dma_start(out=csig, in_=Csig_d[:, :])
            nc.sync.dma_start(out=esig, in_=Esig_d[:, :])
            nc.sync.dma_start(out=dt_t, in_=Dt_d[:, :])
            nc.sync.dma_start(out=bt_t, in_=bT_d[:, :])
            nc.sync.dma_start(out=wb, in_=WBt_d[:, :, :])

            for j in range(KI):
                nc.vector.tensor_scalar_mul(
                    out=cd[:, j, :], in0=csig, scalar1=dt_t[:, j : j + 1]
                )

            tdt = f32 if need_round else DT

            xf = [big1.tile([128, KI, bh], tdt, tag="big1", name=f"xf{h}")
                  for h in range(nsplit)]
            for j in range(KI):
                st = spool.tile([BS, BC], DT, tag="stage")
                nc.sync.dma_start(out=st, in_=xT[:, j, :])
                ps = psA.tile([128, BC], f32, tag="ps", name=f"psa{j}")
                nc.tensor.matmul(ps, cd[:, j, :], st, start=True, stop=True)
                for h in range(nsplit):
                    do_copy(j + h, xf[h][:, j, :], ps[:, h * bh : (h + 1) * bh])

            for h in range(nsplit):
                z = big2.tile([128, 32, bh], tdt, tag="big2", name=f"z{h}")
                nc.vector.transpose(out=z.transpose([0, 2, 1]), in_=xf[h].transpose([0, 2, 1]))
                if need_round:
                    zr = big3.tile([128, 32, bh], DT, tag="big3", name=f"zr{h}")
                    nc.gpsimd.dma_start(out=zr, in_=z)
                    z = zr
                yz = big1.tile([128, 32, bh], tdt, tag="big1", name=f"yz{h}")
                for c in range(32):
                    ps = psB.tile([128, bh], f32, tag="ps", name=f"psb{c}_{h}")
                    nc.tensor.matmul(ps, wb[:, c, :], z[:, c, :], start=True, stop=True)
                    do_copy(c, yz[:, c, :], ps, act_frac=4)
                yw = big2.tile([128, 32, bh], tdt, tag="big2", name=f"yw{h}")
                nc.vector.transpose(out=yw.transpose([0, 2, 1]), in_=yz.transpose([0, 2, 1]))
                if need_round:
                    ywr = big3.tile([128, 32, bh], DT, tag="big3", name=f"ywr{h}")
                    nc.gpsimd.dma_start(out=ywr, in_=yw)
                    yw = ywr
                for i in range(0, KO, 2):
                    ps = psC.tile([128, 2, bh], f32, tag="ps", name=f"psc{i}_{h}")
                    nc.tensor.matmul(
                        ps, esig, yw[:, i : i + 2, :], start=True, stop=True
                    )
                    for d in range(2):
                        oi = opool.tile([BS, bh], f32, tag="o")
                        nc.scalar.activation(
                            out=oi, in_=ps[:, d, :],
                            func=mybir.ActivationFunctionType.Identity,
                            bias=bt_t[:, i + d : i + d + 1],
                        )
                        nc.sync.dma_start(
                            out=outT[i + d, :, h * bh : (h + 1) * bh], in_=oi
                        )

    nc.compile()
    _NC_CACHE[key] = nc
    return nc


def _prep_fft(x, W, D, bias):
    Csig, Esig = _pack_const()
    WBt = _pack_wb(W)
    Dt = np.ascontiguousarray(D.reshape(KI, BS).T)
    bT = np.ascontiguousarray(bias.reshape(KO, BS).T)
    in_maps = []
    for c in range(NCORES):
        xs = x[c * BC : (c + 1) * BC, :]
        xTc = np.ascontiguousarray(xs.reshape(BC, KI, BS).transpose(2, 1, 0))
        im = {"xT": xTc, "Csig": Csig, "WBt": WBt, "Esig": Esig, "Dt": Dt, "bT": bT}
        if LDWOPT:
            im["ldwopt_tag"] = np.zeros((1, 1), dtype=np.float32)
        in_maps.append(im)
    return in_maps


# ------------------------------------------------------------------- driver
def _run(inputs, trace=False):
    x = np.asarray(inputs["x"], dtype=np.float32)
    W = np.asarray(inputs["W"], dtype=np.float32)
    D = np.asarray(inputs["D_bernoulli"], dtype=np.float32)
    bias = np.asarray(inputs["bias"], dtype=np.float32)

    if IMPL == "fft3":
        nc = _build_fft3()
        in_maps = _prep_fft3(x, W, D, bias)
    else:
        nc = _build_fft(MM_DTYPE)
        in_maps = _prep_fft(x, W, D, bias)

    res = run_bass_kernel_spmd(nc, in_maps, list(range(NCORES)), trace=trace)
    out = np.empty((BATCH, D_OUT), dtype=np.float32)
    for c in range(NCORES):
        oT = np.asarray(res.results[c]["outT"], dtype=np.float32)  # [i, t, b]
        out[c * BC : (c + 1) * BC, :] = oT.transpose(2, 0, 1).reshape(BC, D_OUT)
    return out, res


def kernel(**inputs) -> np.ndarray:
    out, _ = _run(inputs, trace=False)
    return out


# revision 15
# speedup vs baseline: 1.9769x; 1.9769x over previous
"""Block-circulant matmul kernel for Trainium2 (8 NeuronCores, data-parallel).

Computes out = (x * D) @ M + bias where M is the 4096x4096 block-circulant
matrix built from W[32, 32, 128] (block (i,j) is C_ij[s,t] = W[i,j,(s-t)%128]).

Sharding: batch (4096) split 8 ways -> 512 rows per core; weights replicated.

Implementation "fft2" (default): 3-stage frequency-domain factorization in
bf16 (fp32 PSUM accumulation). Per core:
 - host folds the Bernoulli diagonal into x and ships xT as bf16;
 - stage A: DFT-as-matmul, ONE shared stationary Csig for all 32 blocks;
 - DVE 32x32 stream-transpose; per-slot block-diag matmul (32 stationaries);
 - DVE transpose; iDFT-as-matmul with shared Esig stationary + bias in the
   PSUM-evacuation copies; bf16 output DMA, host upcasts.
The sigma frequency packing puts the 4 real components of a frequency
pair-slot c at spectrum positions {c, 32+c, 64+c, 96+c} so the
quadrant-local DVE transpose lands rows exactly where the next stage's
matmul needs them.

Legacy impls "fft" (fp32 3-stage) and "dense" kept for fallback via BC_IMPL.
"""

import os
import numpy as np

import concourse.bass as bass
import concourse.mybir as mybir
from concourse import bacc
from concourse.tile import TileContext
from concourse.bass_utils import run_bass_kernel_spmd
import concourse.bass_utils as _bu

# Let walrus overlap LDWEIGHTS with in-flight matmuls.
LDWOPT = os.environ.get("BC_LDWOPT", "0") == "1"
if not getattr(_bu, "_bc_ldwopt_patched", False):
    _bu._bc_ldwopt_patched = True
    _orig_bvo = _bu.bir_verify_and_optimise

    def _bvo_ldwopt(*a, **k):
        orig_rc = _bu.run_command

        def rc(argv, **kw):
            if LDWOPT:
                argv = [s.replace("--enable-ldw-opt=false",
                                  "--enable-ldw-opt=true") for s in argv]
            return orig_rc(argv, **kw)

        _bu.run_command = rc
        try:
            return _orig_bvo(*a, **k)
        finally:
            _bu.run_command = orig_rc

    _bu.bir_verify_and_optimise = _bvo_ldwopt

# Problem constants (hardcoded per harness contract).
BATCH = 4096
D_IN = 4096
D_OUT = 4096
BS = 128          # circulant block size
KI = 32           # input blocks
KO = 32           # output blocks
NCORES = 8
BC = BATCH // NCORES      # 512 batch rows per core
NSPLIT = 2                # batch halves per core (pipeline + PSUM sizing)
BH = BC // NSPLIT

IMPL = os.environ.get("BC_IMPL", "fft3")
MM_DTYPE = os.environ.get("BC_DTYPE", "fp32")   # legacy fft/dense impls only
# engine pattern for each psum-evacuation copy family, cycled per call:
# 'a' = ACT (scalar), 'd' = DVE (vector).  PSUM is not reachable from Pool.
ENG_A = os.environ.get("BC_ENG_A", "a")
ENG_B = os.environ.get("BC_ENG_B", "a")
ENG_C = os.environ.get("BC_ENG_C", "ad")
# bias mechanism: "pe" = 1-partition ones-matmul into PSUM, "evac" = per-i
# Identity+bias during C evacuation
CBIAS = os.environ.get("BC_CBIAS", "pe")
# intermediate dtype: bf16 (fast, known-good) or f32r (fp32 bits, accuracy fallback)
IDT_NAME = os.environ.get("BC_IDT", "bf16")

_NC_CACHE = {}
_PACK_CACHE = {}


def _dt_of(name):
    return {
        "fp32": mybir.dt.float32,
        "f32r": mybir.dt.float32r,
        "bf16": mybir.dt.bfloat16,
    }[name]


def _bf16():
    import ml_dtypes
    return ml_dtypes.bfloat16


# ---------------------------------------------------------------- sigma pack
def _sigma_components():
    """slot c, quadrant Q -> ("re"|"im", f). Pairs (2c+1, 2c+2) for c<31,
    slot 31 holds (63 complex, 0 real, 64 real)."""
    comp = {}
    for c in range(32):
        fa = 2 * c + 1 if c < 31 else 63
        comp[(0, c)] = ("re", fa)
        comp[(1, c)] = ("im", fa)
        if c < 31:
            comp[(2, c)] = ("re", 2 * c + 2)
            comp[(3, c)] = ("im", 2 * c + 2)
        else:
            comp[(2, c)] = ("re", 0)
            comp[(3, c)] = ("re", 64)
    return comp


def _pack_const():
    """Input-independent factor matrices Csig [s, m] and Esig [m, t]."""
    if "const" in _PACK_CACHE:
        return _PACK_CACHE["const"]
    comp = _sigma_components()
    s = np.arange(BS)
    Csig = np.zeros((BS, 128), dtype=np.float64)
    Esig = np.zeros((128, BS), dtype=np.float64)
    for (Q, c), (typ, f) in comp.items():
        m = 32 * Q + c
        ang = 2 * np.pi * f * s / BS
        a = (1.0 if f in (0, 64) else 2.0) / BS
        if typ == "re":
            Csig[:, m] = np.cos(ang)
            Esig[m, :] = a * np.cos(ang)
        else:
            Csig[:, m] = -np.sin(ang)
            Esig[m, :] = -a * np.sin(ang)
    out = (Csig.astype(np.float32), np.ascontiguousarray(Esig.astype(np.float32)))
    _PACK_CACHE["const"] = out
    return out


def _pack_wb(W):
    """Frequency-domain block-diagonal weights WBt [row=(Qr,j), slot, col=(Qc,i)]."""
    comp = _sigma_components()
    Wf = np.fft.fft(W.astype(np.float64), axis=-1)
    Wfr, Wfi = Wf.real, Wf.imag
    WB = np.zeros((32, 128, 128), dtype=np.float64)
    for c in range(32):
        for (qre, qim) in ((0, 1), (2, 3)):
            typ_im = comp[(qim, c)][0]
            f = comp[(qre, c)][1]
            if typ_im == "im":
                wr = Wfr[:, :, f].T  # [j, i]
                wi = Wfi[:, :, f].T
                WB[c, qre*32:(qre+1)*32, qre*32:(qre+1)*32] = wr
                WB[c, qim*32:(qim+1)*32, qre*32:(qre+1)*32] = wi
                WB[c, qre*32:(qre+1)*32, qim*32:(qim+1)*32] = -wi
                WB[c, qim*32:(qim+1)*32, qim*32:(qim+1)*32] = wr
            else:
                f2 = comp[(qim, c)][1]
                WB[c, qre*32:(qre+1)*32, qre*32:(qre+1)*32] = Wfr[:, :, f].T
                WB[c, qim*32:(qim+1)*32, qim*32:(qim+1)*32] = Wfr[:, :, f2].T
    return np.ascontiguousarray(
        WB.transpose(1, 0, 2).astype(np.float32)  # [row, slot, col]
    )


# --------------------------------------------------------------- fft3 build
def _build_fft3():
    key = ("fft3", ENG_A, ENG_B, ENG_C, CBIAS, IDT_NAME)
    if key in _NC_CACHE:
        return _NC_CACHE[key]
    DT = mybir.dt.bfloat16            # x / csig / output dtype
    IDT = _dt_of(IDT_NAME)            # intermediate + B/C weights dtype
    f32 = mybir.dt.float32

    nc = bacc.Bacc(None, target_bir_lowering=False, debug=False)

    xT = nc.dram_tensor("xT", [BS, KI, BC], DT, kind="ExternalInput")
    Csig_d = nc.dram_tensor("Csig", [BS, 128], DT, kind="ExternalInput")
    WBt_d = nc.dram_tensor("WBt", [128, 32, 128], IDT, kind="ExternalInput")
    Esig_d = nc.dram_tensor("Esig", [128, BS], IDT, kind="ExternalInput")
    bTT_d = nc.dram_tensor("bTT", [KO, BS], IDT, kind="ExternalInput")
    mask_d = nc.dram_tensor("mask", [KO, 512], IDT, kind="ExternalInput")
    # out[t, (h,k8), (p,b16), i32] -> host reshapes to [t, b, i]
    outT = nc.dram_tensor("outT", [BS, 16, 4, BH], DT, kind="ExternalOutput")
    if LDWOPT:
        nc.dram_tensor("ldwopt_tag", [1, 1], f32, kind="ExternalInput")

    def eng_iter(pattern):
        n = [0]

        def next_eng():
            e = pattern[n[0] % len(pattern)]
            n[0] += 1
            return e
        return next_eng

    ea, eb, ec = eng_iter(ENG_A), eng_iter(ENG_B), eng_iter(ENG_C)

    def evac(eng, out, in_, bias=None):
        """PSUM -> SBUF evacuation copy on ACT ('a') or DVE ('d')."""
        if bias is not None:
            if eng == "a":
                nc.scalar.activation(
                    out=out, in_=in_,
                    func=mybir.ActivationFunctionType.Identity, bias=bias,
                )
            else:
                nc.vector.tensor_scalar_add(out=out, in0=in_, scalar1=bias)
        else:
            if eng == "a":
                nc.scalar.activation(
                    out=out, in_=in_, func=mybir.ActivationFunctionType.Copy
                )
            else:
                nc.vector.tensor_copy(out=out, in_=in_)

    with TileContext(nc) as tc:
        with tc.tile_pool(name="consts", bufs=1) as cpool, \
             tc.tile_pool(name="stage", bufs=KI) as spool, \
             tc.tile_pool(name="big1", bufs=2) as big1, \
             tc.tile_pool(name="big2", bufs=2) as big2, \
             tc.tile_pool(name="o", bufs=4) as opool, \
             tc.tile_pool(name="psAC", bufs=2, space="PSUM") as psAC, \
             tc.tile_pool(name="psB", bufs=2, space="PSUM") as psB:

            csig = cpool.tile([BS, 128], DT)
            esig = cpool.tile([128, BS], IDT)
            wb = cpool.tile([128, 32, 128], IDT)
            btt = cpool.tile([KO, BS], IDT)
            mask = cpool.tile([KO, 512], IDT)
            nc.sync.dma_start(out=csig, in_=Csig_d[:, :])
            nc.sync.dma_start(out=esig, in_=Esig_d[:, :])
            nc.sync.dma_start(out=wb, in_=WBt_d[:, :, :])
            nc.sync.dma_start(out=btt, in_=bTT_d[:, :])
            nc.sync.dma_start(out=mask, in_=mask_d[:, :])

            # all intermediates b-major: both stream transposes get contiguous
            # input AND output (strided-output transposes run 4x slow)
            xf = [big1.tile([128, BH, KI], IDT, tag="big1", name=f"xf{h}")
                  for h in range(NSPLIT)]
            z = [None] * NSPLIT
            yz = [None] * NSPLIT
            yw = [None] * NSPLIT

            # full-width x staging: one DMA per j, tiles live across halves
            st = [spool.tile([BS, BC], DT, tag="stage", name=f"st{j}")
                  for j in range(KI)]
            for j in range(KI):
                nc.sync.dma_start(out=st[j], in_=xT[:, j, :])

            def stage_a(h):
                # DFT: XF[m, b, j] = sum_s Csig[s, m] * xd[s, j, b]
                for q in range(KI // 4):
                    ps = psAC.tile([128, 4, BH], f32, tag="psAC",
                                   name=f"psa{q}_{h}")
                    for d in range(4):
                        j = 4 * q + d
                        nc.tensor.matmul(
                            ps[:, d, :], csig,
                            st[j][:, h * BH:(h + 1) * BH],
                            start=True, stop=True,
                        )
                    evac(ea(), xf[h][:, :, 4 * q:4 * q + 4],
                         ps.transpose([0, 2, 1]))

            def t1(h):
                # Z[(Q,j), b, c] = XF[(Q,c), b, j]  (contig in AND out)
                z[h] = big2.tile([128, BH, 32], IDT, tag="big2", name=f"z{h}")
                nc.vector.transpose(out=z[h], in_=xf[h])

            def stage_b(h):
                # per-slot block-diagonal frequency matmul (moving stride 32)
                yz[h] = big1.tile([128, BH, 32], IDT, tag="big1", name=f"yz{h}")
                for q in range(8):
                    ps = psB.tile([128, 4, BH], f32, tag="psB",
                                  name=f"psb{q}_{h}")
                    for d in range(4):
                        c = 4 * q + d
                        nc.tensor.matmul(
                            ps[:, d, :], wb[:, c, :], z[h][:, :, c],
                            start=True, stop=True,
                        )
                    evac(eb(), yz[h][:, :, 4 * q:4 * q + 4],
                         ps.transpose([0, 2, 1]))

            def t2(h):
                # YW[(Q,c), b, i] = YZ[(Q,i), b, c]  (contig in AND out)
                yw[h] = big2.tile([128, BH, 32], IDT, tag="big2", name=f"yw{h}")
                nc.vector.transpose(out=yw[h], in_=yz[h])

            def stage_c(h):
                # iDFT over 16-batch chunks: moving yw[:, b16, :] is contiguous,
                # esig stationary shared, bias added exactly via mask matmul.
                for q in range(8):
                    ps = psAC.tile([128, 4, BH], f32, tag="psAC",
                                   name=f"psc{q}_{h}")
                    for p in range(2):
                        bsl = slice(32 * q + 16 * p, 32 * q + 16 * p + 16)
                        nc.tensor.matmul(
                            ps[:, 2 * p:2 * p + 2, :], esig, yw[h][:, bsl, :],
                            start=True, stop=False,
                        )
                        nc.tensor.matmul(
                            ps[:, 2 * p:2 * p + 2, :], btt, mask,
                            start=False, stop=True, skip_group_check=True,
                        )
                    ob = opool.tile([128, 4, BH], DT, tag="o")
                    evac(ec(), ob, ps)
                    nc.scalar.dma_start(
                        out=outT[:, 8 * h + q, :, :], in_=ob
                    )

            # dataflow-ordered emission for pipelined queues
            stage_a(0)
            t1(0)
            stage_a(1)
            stage_b(0)
            t1(1)
            t2(0)
            stage_b(1)
            stage_c(0)
            t2(1)
            stage_c(1)

    nc.compile()
    _NC_CACHE[key] = nc
    return nc


def _prep_fft3(x, W, D, bias):
    bf = _bf16()
    Csig, Esig = _pack_const()
    WBt = _pack_wb(W)
    Csig16 = Csig.astype(bf)
    wdt = bf if IDT_NAME == "bf16" else np.float32
    Esig_w = np.ascontiguousarray(Esig).astype(wdt)
    WBt_w = WBt.astype(wdt)
    bTT = np.ascontiguousarray(bias.reshape(KO, BS)).astype(wdt)
    msk = np.zeros((KO, 512), dtype=np.float32)
    for n in range(512):
        msk[n % 32, n] = 1.0
    msk_w = msk.astype(wdt)
    xd = (x * D[None, :]).astype(np.float32)
    in_maps = []
    for c in range(NCORES):
        xs = xd[c * BC:(c + 1) * BC, :]
        xTc = np.ascontiguousarray(
            xs.reshape(BC, KI, BS).transpose(2, 1, 0)
        ).astype(bf)
        im = {"xT": xTc, "Csig": Csig16, "WBt": WBt_w, "Esig": Esig_w,
              "bTT": bTT, "mask": msk_w}
        if LDWOPT:
            im["ldwopt_tag"] = np.zeros((1, 1), dtype=np.float32)
        in_maps.append(im)
    return in_maps


# ---------------------------------------------------------------- fft build
def _build_fft(mm_dtype):
    key = ("fft", mm_dtype)
    if key in _NC_CACHE:
        return _NC_CACHE[key]
    DT = _dt_of(mm_dtype)
    f32 = mybir.dt.float32
    need_round = DT == mybir.dt.float32r
    nsplit = 2
    bh = BC // nsplit

    nc = bacc.Bacc(None, target_bir_lowering=False, debug=False)

    xT = nc.dram_tensor("xT", [BS, KI, BC], DT, kind="ExternalInput")
    Csig_d = nc.dram_tensor("Csig", [BS, 128], f32, kind="ExternalInput")
    WBt_d = nc.dram_tensor("WBt", [128, 32, 128], DT, kind="ExternalInput")
    Esig_d = nc.dram_tensor("Esig", [128, BS], DT, kind="ExternalInput")
    Dt_d = nc.dram_tensor("Dt", [BS, KI], f32, kind="ExternalInput")
    bT_d = nc.dram_tensor("bT", [BS, KO], f32, kind="ExternalInput")
    outT = nc.dram_tensor("outT", [KO, BS, BC], f32, kind="ExternalOutput")
    if LDWOPT:
        nc.dram_tensor("ldwopt_tag", [1, 1], f32, kind="ExternalInput")

    def do_copy(k, out, in_, act_frac=2):
        if k % act_frac == 0:
            nc.vector.tensor_copy(out=out, in_=in_)
        else:
            nc.scalar.activation(
                out=out, in_=in_, func=mybir.ActivationFunctionType.Copy
            )

    with TileContext(nc) as tc:
        with tc.tile_pool(name="consts", bufs=1) as cpool, \
             tc.tile_pool(name="stage", bufs=6) as spool, \
             tc.tile_pool(name="big1", bufs=2) as big1, \
             tc.tile_pool(name="big2", bufs=2) as big2, \
             tc.tile_pool(name="big3", bufs=1) as big3, \
             tc.tile_pool(name="o", bufs=6) as opool, \
             tc.tile_pool(name="psAll", bufs=8, space="PSUM") as psAll:

            psA = psB = psC = psAll
            csig = cpool.tile([BS, 128], f32)
            esig = cpool.tile([128, BS], DT)
            dt_t = cpool.tile([BS, KI], f32)
            bt_t = cpool.tile([BS, KO], f32)
            wb = cpool.tile([128, 32, 128], DT)
            if need_round:
                cd = big3.tile([BS, KI, 128], DT, tag="big3", name="cd")
            else:
                cd = cpool.tile([BS, KI, 128], DT)
            nc.sync.dma_start(out=csig, in_=Csig_d[:, :])
            nc.sync.dma_start(out=esig, in_=Esig_d[:, :])
            nc.sync.dma_start(out=dt_t, in_=Dt_d[:, :])
            nc.sync.dma_start(out=bt_t, in_=bT_d[:, :])
            nc.sync.dma_start(out=wb, in_=WBt_d[:, :, :])

            for j in range(KI):
                nc.vector.tensor_scalar_mul(
                    out=cd[:, j, :], in0=csig, scalar1=dt_t[:, j : j + 1]
                )

            tdt = f32 if need_round else DT

            xf = [big1.tile([128, KI, bh], tdt, tag="big1", name=f"xf{h}")
                  for h in range(nsplit)]
            for j in range(KI):
                st = spool.tile([BS, BC], DT, tag="stage")
                nc.sync.dma_start(out=st, in_=xT[:, j, :])
                ps = psA.tile([128, BC], f32, tag="ps", name=f"psa{j}")
                nc.tensor.matmul(ps, cd[:, j, :], st, start=True, stop=True)
                for h in range(nsplit):
                    do_copy(j + h, xf[h][:, j, :], ps[:, h * bh : (h + 1) * bh])

            for h in range(nsplit):
                z = big2.tile([128, 32, bh], tdt, tag="big2", name=f"z{h}")
                nc.vector.transpose(out=z.transpose([0, 2, 1]), in_=xf[h].transpose([0, 2, 1]))
                if need_round:
                    zr = big3.tile([128, 32, bh], DT, tag="big3", name=f"zr{h}")
                    nc.gpsimd.dma_start(out=zr, in_=z)
                    z = zr
                yz = big1.tile([128, 32, bh], tdt, tag="big1", name=f"yz{h}")
                for c in range(32):
                    ps = psB.tile([128, bh], f32, tag="ps", name=f"psb{c}_{h}")
                    nc.tensor.matmul(ps, wb[:, c, :], z[:, c, :], start=True, stop=True)
                    do_copy(c, yz[:, c, :], ps, act_frac=4)
                yw = big2.tile([128, 32, bh], tdt, tag="big2", name=f"yw{h}")
                nc.vector.transpose(out=yw.transpose([0, 2, 1]), in_=yz.transpose([0, 2, 1]))
                if need_round:
                    ywr = big3.tile([128, 32, bh], DT, tag="big3", name=f"ywr{h}")
                    nc.gpsimd.dma_start(out=ywr, in_=yw)
                    yw = ywr
                for i in range(0, KO, 2):
                    ps = psC.tile([128, 2, bh], f32, tag="ps", name=f"psc{i}_{h}")
                    nc.tensor.matmul(
                        ps, esig, yw[:, i : i + 2, :], start=True, stop=True
                    )
                    for d in range(2):
                        oi = opool.tile([BS, bh], f32, tag="o")
                        nc.scalar.activation(
                            out=oi, in_=ps[:, d, :],
                            func=mybir.ActivationFunctionType.Identity,
                            bias=bt_t[:, i + d : i + d + 1],
                        )
                        nc.sync.dma_start(
                            out=outT[i + d, :, h * bh : (h + 1) * bh], in_=oi
                        )

    nc.compile()
    _NC_CACHE[key] = nc
    return nc


def _prep_fft(x, W, D, bias):
    Csig, Esig = _pack_const()
    WBt = _pack_wb(W)
    Dt = np.ascontiguousarray(D.reshape(KI, BS).T)
    bT = np.ascontiguousarray(bias.reshape(KO, BS).T)
    in_maps = []
    for c in range(NCORES):
        xs = x[c * BC : (c + 1) * BC, :]
        xTc = np.ascontiguousarray(xs.reshape(BC, KI, BS).transpose(2, 1, 0))
        im = {"xT": xTc, "Csig": Csig, "WBt": WBt, "Esig": Esig, "Dt": Dt, "bT": bT}
        if LDWOPT:
            im["ldwopt_tag"] = np.zeros((1, 1), dtype=np.float32)
        in_maps.append(im)
    return in_maps


# ------------------------------------------------------------------- driver
def _run(inputs, trace=False):
    x = np.asarray(inputs["x"], dtype=np.float32)
    W = np.asarray(inputs["W"], dtype=np.float32)
    D = np.asarray(inputs["D_bernoulli"], dtype=np.float32)
    bias = np.asarray(inputs["bias"], dtype=np.float32)

    if IMPL == "fft3":
        nc = _build_fft3()
        in_maps = _prep_fft3(x, W, D, bias)
    else:
        nc = _build_fft(MM_DTYPE)
        in_maps = _prep_fft(x, W, D, bias)

    res = run_bass_kernel_spmd(nc, in_maps, list(range(NCORES)), trace=trace)
    out = np.empty((BATCH, D_OUT), dtype=np.float32)
    for c in range(NCORES):
        oT = np.asarray(res.results[c]["outT"], dtype=np.float32)
        if IMPL == "fft3":
            # [t, (h,q), (p, b16, i)] -> [b, i*128+t]
            ob = oT.reshape(BS, 16, 2, 16, KO).transpose(1, 2, 3, 4, 0)
            out[c * BC : (c + 1) * BC, :] = ob.reshape(BC, D_OUT)
        else:
            # [i, t, b]
            out[c * BC : (c + 1) * BC, :] = (
                oT.transpose(2, 0, 1).reshape(BC, D_OUT)
            )
    return out, res


def kernel(**inputs) -> np.ndarray:
    out, _ = _run(inputs, trace=False)
    return out


# revision 17
# speedup vs baseline: 2.0516x; 1.0377x over previous
"""Block-circulant matmul kernel for Trainium2 (8 NeuronCores, data-parallel).

Computes out = (x * D) @ M + bias where M is the 4096x4096 block-circulant
matrix built from W[32, 32, 128] (block (i,j) is C_ij[s,t] = W[i,j,(s-t)%128]).

Sharding: batch (4096) split 8 ways -> 512 rows per core; weights replicated.

Implementation "fft2" (default): 3-stage frequency-domain factorization in
bf16 (fp32 PSUM accumulation). Per core:
 - host folds the Bernoulli diagonal into x and ships xT as bf16;
 - stage A: DFT-as-matmul, ONE shared stationary Csig for all 32 blocks;
 - DVE 32x32 stream-transpose; per-slot block-diag matmul (32 stationaries);
 - DVE transpose; iDFT-as-matmul with shared Esig stationary + bias in the
   PSUM-evacuation copies; bf16 output DMA, host upcasts.
The sigma frequency packing puts the 4 real components of a frequency
pair-slot c at spectrum positions {c, 32+c, 64+c, 96+c} so the
quadrant-local DVE transpose lands rows exactly where the next stage's
matmul needs them.

Legacy impls "fft" (fp32 3-stage) and "dense" kept for fallback via BC_IMPL.
"""

import os
import numpy as np

import concourse.bass as bass
import concourse.mybir as mybir
from concourse import bacc
from concourse.tile import TileContext
from concourse.bass_utils import run_bass_kernel_spmd
import concourse.bass_utils as _bu

# Let walrus overlap LDWEIGHTS with in-flight matmuls.
LDWOPT = os.environ.get("BC_LDWOPT", "0") == "1"
if not getattr(_bu, "_bc_ldwopt_patched", False):
    _bu._bc_ldwopt_patched = True
    _orig_bvo = _bu.bir_verify_and_optimise

    def _bvo_ldwopt(*a, **k):
        orig_rc = _bu.run_command

        def rc(argv, **kw):
            if LDWOPT:
                argv = [s.replace("--enable-ldw-opt=false",
                                  "--enable-ldw-opt=true") for s in argv]
            return orig_rc(argv, **kw)

        _bu.run_command = rc
        try:
            return _orig_bvo(*a, **k)
        finally:
            _bu.run_command = orig_rc

    _bu.bir_verify_and_optimise = _bvo_ldwopt

# Problem constants (hardcoded per harness contract).
BATCH = 4096
D_IN = 4096
D_OUT = 4096
BS = 128          # circulant block size
KI = 32           # input blocks
KO = 32           # output blocks
NCORES = 8
BC = BATCH // NCORES      # 512 batch rows per core
NSPLIT = 2                # batch halves per core (pipeline + PSUM sizing)
BH = BC // NSPLIT

IMPL = os.environ.get("BC_IMPL", "fft3")
MM_DTYPE = os.environ.get("BC_DTYPE", "fp32")   # legacy fft/dense impls only
# engine pattern for each psum-evacuation copy family, cycled per call:
# 'a' = ACT (scalar), 'd' = DVE (vector).  PSUM is not reachable from Pool.
ENG_A = os.environ.get("BC_ENG_A", "a")
ENG_B = os.environ.get("BC_ENG_B", "a")
ENG_C = os.environ.get("BC_ENG_C", "ad")
# bias mechanism: "pe" = 1-partition ones-matmul into PSUM, "evac" = per-i
# Identity+bias during C evacuation
CBIAS = os.environ.get("BC_CBIAS", "pe")
# intermediate dtype: bf16 (fast, known-good) or f32r (fp32 bits, accuracy fallback)
IDT_NAME = os.environ.get("BC_IDT", "bf16")

_NC_CACHE = {}
_PACK_CACHE = {}


def _dt_of(name):
    return {
        "fp32": mybir.dt.float32,
        "f32r": mybir.dt.float32r,
        "bf16": mybir.dt.bfloat16,
    }[name]


def _bf16():
    import ml_dtypes
    return ml_dtypes.bfloat16


# ---------------------------------------------------------------- sigma pack
def _sigma_components():
    """slot c, quadrant Q -> ("re"|"im", f). Pairs (2c+1, 2c+2) for c<31,
    slot 31 holds (63 complex, 0 real, 64 real)."""
    comp = {}
    for c in range(32):
        fa = 2 * c + 1 if c < 31 else 63
        comp[(0, c)] = ("re", fa)
        comp[(1, c)] = ("im", fa)
        if c < 31:
            comp[(2, c)] = ("re", 2 * c + 2)
            comp[(3, c)] = ("im", 2 * c + 2)
        else:
            comp[(2, c)] = ("re", 0)
            comp[(3, c)] = ("re", 64)
    return comp


def _pack_const():
    """Input-independent factor matrices Csig [s, m] and Esig [m, t]."""
    if "const" in _PACK_CACHE:
        return _PACK_CACHE["const"]
    comp = _sigma_components()
    s = np.arange(BS)
    Csig = np.zeros((BS, 128), dtype=np.float64)
    Esig = np.zeros((128, BS), dtype=np.float64)
    for (Q, c), (typ, f) in comp.items():
        m = 32 * Q + c
        ang = 2 * np.pi * f * s / BS
        a = (1.0 if f in (0, 64) else 2.0) / BS
        if typ == "re":
            Csig[:, m] = np.cos(ang)
            Esig[m, :] = a * np.cos(ang)
        else:
            Csig[:, m] = -np.sin(ang)
            Esig[m, :] = -a * np.sin(ang)
    out = (Csig.astype(np.float32), np.ascontiguousarray(Esig.astype(np.float32)))
    _PACK_CACHE["const"] = out
    return out


def _pack_wb(W):
    """Frequency-domain block-diagonal weights WBt [row=(Qr,j), slot, col=(Qc,i)]."""
    comp = _sigma_components()
    Wf = np.fft.fft(W.astype(np.float64), axis=-1)
    Wfr, Wfi = Wf.real, Wf.imag
    WB = np.zeros((32, 128, 128), dtype=np.float64)
    for c in range(32):
        for (qre, qim) in ((0, 1), (2, 3)):
            typ_im = comp[(qim, c)][0]
            f = comp[(qre, c)][1]
            if typ_im == "im":
                wr = Wfr[:, :, f].T  # [j, i]
                wi = Wfi[:, :, f].T
                WB[c, qre*32:(qre+1)*32, qre*32:(qre+1)*32] = wr
                WB[c, qim*32:(qim+1)*32, qre*32:(qre+1)*32] = wi
                WB[c, qre*32:(qre+1)*32, qim*32:(qim+1)*32] = -wi
                WB[c, qim*32:(qim+1)*32, qim*32:(qim+1)*32] = wr
            else:
                f2 = comp[(qim, c)][1]
                WB[c, qre*32:(qre+1)*32, qre*32:(qre+1)*32] = Wfr[:, :, f].T
                WB[c, qim*32:(qim+1)*32, qim*32:(qim+1)*32] = Wfr[:, :, f2].T
    return np.ascontiguousarray(
        WB.transpose(1, 0, 2).astype(np.float32)  # [row, slot, col]
    )


# --------------------------------------------------------------- fft3 build
def _build_fft3():
    key = ("fft3", ENG_A, ENG_B, ENG_C, CBIAS, IDT_NAME)
    if key in _NC_CACHE:
        return _NC_CACHE[key]
    DT = mybir.dt.bfloat16            # x / csig / output dtype
    IDT = _dt_of(IDT_NAME)            # intermediate + B/C weights dtype
    f32 = mybir.dt.float32

    nc = bacc.Bacc(None, target_bir_lowering=False, debug=False)

    xT = nc.dram_tensor("xT", [BS, KI, BC], DT, kind="ExternalInput")
    Csig_d = nc.dram_tensor("Csig", [BS, 128], DT, kind="ExternalInput")
    WBt_d = nc.dram_tensor("WBt", [128, 32, 128], IDT, kind="ExternalInput")
    Esig_d = nc.dram_tensor("Esig", [128, BS], IDT, kind="ExternalInput")
    bTT_d = nc.dram_tensor("bTT", [KO, BS], IDT, kind="ExternalInput")
    mask_d = nc.dram_tensor("mask", [KO, 512], IDT, kind="ExternalInput")
    # out[t, (h,k8), (p,b16), i32] -> host reshapes to [t, b, i]
    outT = nc.dram_tensor("outT", [BS, 16, 4, BH], DT, kind="ExternalOutput")
    if LDWOPT:
        nc.dram_tensor("ldwopt_tag", [1, 1], f32, kind="ExternalInput")

    def eng_iter(pattern):
        n = [0]

        def next_eng():
            e = pattern[n[0] % len(pattern)]
            n[0] += 1
            return e
        return next_eng

    ea, eb, ec = eng_iter(ENG_A), eng_iter(ENG_B), eng_iter(ENG_C)

    def evac(eng, out, in_, bias=None):
        """PSUM -> SBUF evacuation copy on ACT ('a') or DVE ('d')."""
        if bias is not None:
            if eng == "a":
                nc.scalar.activation(
                    out=out, in_=in_,
                    func=mybir.ActivationFunctionType.Identity, bias=bias,
                )
            else:
                nc.vector.tensor_scalar_add(out=out, in0=in_, scalar1=bias)
        else:
            if eng == "a":
                nc.scalar.activation(
                    out=out, in_=in_, func=mybir.ActivationFunctionType.Copy
                )
            else:
                nc.vector.tensor_copy(out=out, in_=in_)

    with TileContext(nc) as tc:
        with tc.tile_pool(name="consts", bufs=1) as cpool, \
             tc.tile_pool(name="stage", bufs=KI) as spool, \
             tc.tile_pool(name="big1", bufs=2) as big1, \
             tc.tile_pool(name="big2", bufs=2) as big2, \
             tc.tile_pool(name="o", bufs=4) as opool, \
             tc.tile_pool(name="ps", bufs=4, space="PSUM") as pspool:

            csig = cpool.tile([BS, 128], DT)
            esig = cpool.tile([128, BS], IDT)
            wb = cpool.tile([128, 32, 128], IDT)
            btt = cpool.tile([KO, BS], IDT)
            mask = cpool.tile([KO, 512], IDT)
            nc.sync.dma_start(out=csig, in_=Csig_d[:, :])
            nc.sync.dma_start(out=esig, in_=Esig_d[:, :])
            nc.sync.dma_start(out=wb, in_=WBt_d[:, :, :])
            nc.sync.dma_start(out=btt, in_=bTT_d[:, :])
            nc.sync.dma_start(out=mask, in_=mask_d[:, :])

            # all intermediates b-major: both stream transposes get contiguous
            # input AND output (strided-output transposes run 4x slow)
            xf = [big1.tile([128, BH, KI], IDT, tag="big1", name=f"xf{h}")
                  for h in range(NSPLIT)]
            z = [None] * NSPLIT
            yz = [None] * NSPLIT
            yw = [None] * NSPLIT

            # full-width x staging: one DMA per j, tiles live across halves;
            # loads are emitted inside stage_a(0) so the first matmuls are not
            # gated on the whole input transfer
            st = [spool.tile([BS, BC], DT, tag="stage", name=f"st{j}")
                  for j in range(KI)]

            def stage_a(h):
                # DFT: XF[m, b, j] = sum_s Csig[s, m] * xd[s, j, b]
                for q in range(KI // 4):
                    if h == 0:
                        for d in range(4):
                            j = 4 * q + d
                            nc.sync.dma_start(out=st[j], in_=xT[:, j, :])
                    ps = pspool.tile([128, 4, BH], f32, tag="ps",
                                     name=f"psa{q}_{h}")
                    for d in range(4):
                        j = 4 * q + d
                        nc.tensor.matmul(
                            ps[:, d, :], csig,
                            st[j][:, h * BH:(h + 1) * BH],
                            start=True, stop=True,
                        )
                    evac(ea(), xf[h][:, :, 4 * q:4 * q + 4],
                         ps.transpose([0, 2, 1]))

            def t1(h):
                # Z[(Q,j), b, c] = XF[(Q,c), b, j]  (contig in AND out)
                z[h] = big2.tile([128, BH, 32], IDT, tag="big2", name=f"z{h}")
                nc.vector.transpose(out=z[h], in_=xf[h])

            def stage_b(h):
                # per-slot block-diagonal frequency matmul (moving stride 32)
                yz[h] = big1.tile([128, BH, 32], IDT, tag="big1", name=f"yz{h}")
                for q in range(8):
                    ps = pspool.tile([128, 4, BH], f32, tag="ps",
                                     name=f"psb{q}_{h}")
                    for d in range(4):
                        c = 4 * q + d
                        nc.tensor.matmul(
                            ps[:, d, :], wb[:, c, :], z[h][:, :, c],
                            start=True, stop=True,
                        )
                    evac(eb(), yz[h][:, :, 4 * q:4 * q + 4],
                         ps.transpose([0, 2, 1]))

            def t2(h):
                # YW[(Q,c), b, i] = YZ[(Q,i), b, c]  (contig in AND out)
                yw[h] = big2.tile([128, BH, 32], IDT, tag="big2", name=f"yw{h}")
                nc.vector.transpose(out=yw[h], in_=yz[h])

            def stage_c(h):
                # iDFT over 16-batch chunks: moving yw[:, b16, :] is contiguous,
                # esig stationary shared, bias added exactly via mask matmul.
                for q in range(8):
                    ps = pspool.tile([128, 4, BH], f32, tag="ps",
                                     name=f"psc{q}_{h}")
                    for p in range(2):
                        bsl = slice(32 * q + 16 * p, 32 * q + 16 * p + 16)
                        nc.tensor.matmul(
                            ps[:, 2 * p:2 * p + 2, :], esig, yw[h][:, bsl, :],
                            start=True, stop=False,
                        )
                        nc.tensor.matmul(
                            ps[:, 2 * p:2 * p + 2, :], btt, mask,
                            start=False, stop=True, skip_group_check=True,
                        )
                    ob = opool.tile([128, 4, BH], DT, tag="o")
                    evac(ec(), ob, ps)
                    nc.scalar.dma_start(
                        out=outT[:, 8 * h + q, :, :], in_=ob
                    )

            # dataflow-ordered emission for pipelined queues
            stage_a(0)
            t1(0)
            stage_a(1)
            stage_b(0)
            t1(1)
            t2(0)
            stage_b(1)
            stage_c(0)
            t2(1)
            stage_c(1)

    nc.compile()
    _NC_CACHE[key] = nc
    return nc


def _prep_fft3(x, W, D, bias):
    bf = _bf16()
    Csig, Esig = _pack_const()
    WBt = _pack_wb(W)
    Csig16 = Csig.astype(bf)
    wdt = bf if IDT_NAME == "bf16" else np.float32
    Esig_w = np.ascontiguousarray(Esig).astype(wdt)
    WBt_w = WBt.astype(wdt)
    bTT = np.ascontiguousarray(bias.reshape(KO, BS)).astype(wdt)
    msk = np.zeros((KO, 512), dtype=np.float32)
    for n in range(512):
        msk[n % 32, n] = 1.0
    msk_w = msk.astype(wdt)
    xd = (x * D[None, :]).astype(np.float32)
    in_maps = []
    for c in range(NCORES):
        xs = xd[c * BC:(c + 1) * BC, :]
        xTc = np.ascontiguousarray(
            xs.reshape(BC, KI, BS).transpose(2, 1, 0)
        ).astype(bf)
        im = {"xT": xTc, "Csig": Csig16, "WBt": WBt_w, "Esig": Esig_w,
              "bTT": bTT, "mask": msk_w}
        if LDWOPT:
            im["ldwopt_tag"] = np.zeros((1, 1), dtype=np.float32)
        in_maps.append(im)
    return in_maps


# ---------------------------------------------------------------- fft build
def _build_fft(mm_dtype):
    key = ("fft", mm_dtype)
    if key in _NC_CACHE:
        return _NC_CACHE[key]
    DT = _dt_of(mm_dtype)
    f32 = mybir.dt.float32
    need_round = DT == mybir.dt.float32r
    nsplit = 2
    bh = BC // nsplit

    nc = bacc.Bacc(None, target_bir_lowering=False, debug=False)

    xT = nc.dram_tensor("xT", [BS, KI, BC], DT, kind="ExternalInput")
    Csig_d = nc.dram_tensor("Csig", [BS, 128], f32, kind="ExternalInput")
    WBt_d = nc.dram_tensor("WBt", [128, 32, 128], DT, kind="ExternalInput")
    Esig_d = nc.dram_tensor("Esig", [128, BS], DT, kind="ExternalInput")
    Dt_d = nc.dram_tensor("Dt", [BS, KI], f32, kind="ExternalInput")
    bT_d = nc.dram_tensor("bT", [BS, KO], f32, kind="ExternalInput")
    outT = nc.dram_tensor("outT", [KO, BS, BC], f32, kind="ExternalOutput")
    if LDWOPT:
        nc.dram_tensor("ldwopt_tag", [1, 1], f32, kind="ExternalInput")

    def do_copy(k, out, in_, act_frac=2):
        if k % act_frac == 0:
            nc.vector.tensor_copy(out=out, in_=in_)
        else:
            nc.scalar.activation(
                out=out, in_=in_, func=mybir.ActivationFunctionType.Copy
            )

    with TileContext(nc) as tc:
        with tc.tile_pool(name="consts", bufs=1) as cpool, \
             tc.tile_pool(name="stage", bufs=6) as spool, \
             tc.tile_pool(name="big1", bufs=2) as big1, \
             tc.tile_pool(name="big2", bufs=2) as big2, \
             tc.tile_pool(name="big3", bufs=1) as big3, \
             tc.tile_pool(name="o", bufs=6) as opool, \
             tc.tile_pool(name="psAll", bufs=8, space="PSUM") as psAll:

            psA = psB = psC = psAll
            csig = cpool.tile([BS, 128], f32)
            esig = cpool.tile([128, BS], DT)
            dt_t = cpool.tile([BS, KI], f32)
            bt_t = cpool.tile([BS, KO], f32)
            wb = cpool.tile([128, 32, 128], DT)
            if need_round:
                cd = big3.tile([BS, KI, 128], DT, tag="big3", name="cd")
            else:
                cd = cpool.tile([BS, KI, 128], DT)
            nc.sync.dma_start(out=csig, in_=Csig_d[:, :])
            nc.sync.dma_start(out=esig, in_=Esig_d[:, :])
            nc.sync.dma_start(out=dt_t, in_=Dt_d[:, :])
            nc.sync.dma_start(out=bt_t, in_=bT_d[:, :])
            nc.sync.dma_start(out=wb, in_=WBt_d[:, :, :])

            for j in range(KI):
                nc.vector.tensor_scalar_mul(
                    out=cd[:, j, :], in0=csig, scalar1=dt_t[:, j : j + 1]
                )

            tdt = f32 if need_round else DT

            xf = [big1.tile([128, KI, bh], tdt, tag="big1", name=f"xf{h}")
                  for h in range(nsplit)]
            for j in range(KI):
                st = spool.tile([BS, BC], DT, tag="stage")
                nc.sync.dma_start(out=st, in_=xT[:, j, :])
                ps = psA.tile([128, BC], f32, tag="ps", name=f"psa{j}")
                nc.tensor.matmul(ps, cd[:, j, :], st, start=True, stop=True)
                for h in range(nsplit):
                    do_copy(j + h, xf[h][:, j, :], ps[:, h * bh : (h + 1) * bh])

            for h in range(nsplit):
                z = big2.tile([128, 32, bh], tdt, tag="big2", name=f"z{h}")
                nc.vector.transpose(out=z.transpose([0, 2, 1]), in_=xf[h].transpose([0, 2, 1]))
                if need_round:
                    zr = big3.tile([128, 32, bh], DT, tag="big3", name=f"zr{h}")
                    nc.gpsimd.dma_start(out=zr, in_=z)
                    z = zr
                yz = big1.tile([128, 32, bh], tdt, tag="big1", name=f"yz{h}")
                for c in range(32):
                    ps = psB.tile([128, bh], f32, tag="ps", name=f"psb{c}_{h}")
                    nc.tensor.matmul(ps, wb[:, c, :], z[:, c, :], start=True, stop=True)
                    do_copy(c, yz[:, c, :], ps, act_frac=4)
                yw = big2.tile([128, 32, bh], tdt, tag="big2", name=f"yw{h}")
                nc.vector.transpose(out=yw.transpose([0, 2, 1]), in_=yz.transpose([0, 2, 1]))
                if need_round:
                    ywr = big3.tile([128, 32, bh], DT, tag="big3", name=f"ywr{h}")
                    nc.gpsimd.dma_start(out=ywr, in_=yw)
                    yw = ywr
                for i in range(0, KO, 2):
                    ps = psC.tile([128, 2, bh], f32, tag="ps", name=f"psc{i}_{h}")
                    nc.tensor.matmul(
                        ps, esig, yw[:, i : i + 2, :], start=True, stop=True
                    )
                    for d in range(2):
                        oi = opool.tile([BS, bh], f32, tag="o")
                        nc.scalar.activation(
                            out=oi, in_=ps[:, d, :],
                            func=mybir.ActivationFunctionType.Identity,
                            bias=bt_t[:, i + d : i + d + 1],
                        )
                        nc.sync.dma_start(
                            out=outT[i + d, :, h * bh : (h + 1) * bh], in_=oi
                        )

    nc.compile()
    _NC_CACHE[key] = nc
    return nc


def _prep_fft(x, W, D, bias):
    Csig, Esig = _pack_const()
    WBt = _pack_wb(W)
    Dt = np.ascontiguousarray(D.reshape(KI, BS).T)
    bT = np.ascontiguousarray(bias.reshape(KO, BS).T)
    in_maps = []
    for c in range(NCORES):
        xs = x[c * BC : (c + 1) * BC, :]
        xTc = np.ascontiguousarray(xs.reshape(BC, KI, BS).transpose(2, 1, 0))
        im = {"xT": xTc, "Csig": Csig, "WBt": WBt, "Esig": Esig, "Dt": Dt, "bT": bT}
        if LDWOPT:
            im["ldwopt_tag"] = np.zeros((1, 1), dtype=np.float32)
        in_maps.append(im)
    return in_maps


# ------------------------------------------------------------------- driver
def _run(inputs, trace=False):
    x = np.asarray(inputs["x"], dtype=np.float32)
    W = np.asarray(inputs["W"], dtype=np.float32)
    D = np.asarray(inputs["D_bernoulli"], dtype=np.float32)
    bias = np.asarray(inputs["bias"], dtype=np.float32)

    if IMPL == "fft3":
        nc = _build_fft3()
        in_maps = _prep_fft3(x, W, D, bias)
    else:
        nc = _build_fft(MM_DTYPE)
        in_maps = _prep_fft(x, W, D, bias)

    res = run_bass_kernel_spmd(nc, in_maps, list(range(NCORES)), trace=trace)
    out = np.empty((BATCH, D_OUT), dtype=np.float32)
    for c in range(NCORES):
        oT = np.asarray(res.results[c]["outT"], dtype=np.float32)
        if IMPL == "fft3":
            # [t, (h,q), (p, b16, i)] -> [b, i*128+t]
            ob = oT.reshape(BS, 16, 2, 16, KO).transpose(1, 2, 3, 4, 0)
            out[c * BC : (c + 1) * BC, :] = ob.reshape(BC, D_OUT)
        else:
            # [i, t, b]
            out[c * BC : (c + 1) * BC, :] = (
                oT.transpose(2, 0, 1).reshape(BC, D_OUT)
            )
    return out, res


def kernel(**inputs) -> np.ndarray:
    out, _ = _run(inputs, trace=False)
    return out


# revision 20
# speedup vs baseline: 2.0851x; 1.0164x over previous
"""Block-circulant matmul kernel for Trainium2 (8 NeuronCores, data-parallel).

Computes out = (x * D) @ M + bias where M is the 4096x4096 block-circulant
matrix built from W[32, 32, 128] (block (i,j) is C_ij[s,t] = W[i,j,(s-t)%128]).

Sharding: batch (4096) split 8 ways -> 512 rows per core; weights replicated.

Implementation "fft2" (default): 3-stage frequency-domain factorization in
bf16 (fp32 PSUM accumulation). Per core:
 - host folds the Bernoulli diagonal into x and ships xT as bf16;
 - stage A: DFT-as-matmul, ONE shared stationary Csig for all 32 blocks;
 - DVE 32x32 stream-transpose; per-slot block-diag matmul (32 stationaries);
 - DVE transpose; iDFT-as-matmul with shared Esig stationary + bias in the
   PSUM-evacuation copies; bf16 output DMA, host upcasts.
The sigma frequency packing puts the 4 real components of a frequency
pair-slot c at spectrum positions {c, 32+c, 64+c, 96+c} so the
quadrant-local DVE transpose lands rows exactly where the next stage's
matmul needs them.

Legacy impls "fft" (fp32 3-stage) and "dense" kept for fallback via BC_IMPL.
"""

import os
import numpy as np

import concourse.bass as bass
import concourse.mybir as mybir
from concourse import bacc
from concourse.tile import TileContext
from concourse.bass_utils import run_bass_kernel_spmd
import concourse.bass_utils as _bu

# Let walrus overlap LDWEIGHTS with in-flight matmuls.
LDWOPT = os.environ.get("BC_LDWOPT", "0") == "1"
if not getattr(_bu, "_bc_ldwopt_patched", False):
    _bu._bc_ldwopt_patched = True
    _orig_bvo = _bu.bir_verify_and_optimise

    def _bvo_ldwopt(*a, **k):
        orig_rc = _bu.run_command

        def rc(argv, **kw):
            if LDWOPT:
                argv = [s.replace("--enable-ldw-opt=false",
                                  "--enable-ldw-opt=true") for s in argv]
            return orig_rc(argv, **kw)

        _bu.run_command = rc
        try:
            return _orig_bvo(*a, **k)
        finally:
            _bu.run_command = orig_rc

    _bu.bir_verify_and_optimise = _bvo_ldwopt

# Problem constants (hardcoded per harness contract).
BATCH = 4096
D_IN = 4096
D_OUT = 4096
BS = 128          # circulant block size
KI = 32           # input blocks
KO = 32           # output blocks
NCORES = 8
BC = BATCH // NCORES      # 512 batch rows per core
NSPLIT = 2                # batch halves per core (pipeline + PSUM sizing)
BH = BC // NSPLIT

IMPL = os.environ.get("BC_IMPL", "fft3")
MM_DTYPE = os.environ.get("BC_DTYPE", "fp32")   # legacy fft/dense impls only
# engine pattern for each psum-evacuation copy family, cycled per call:
# 'a' = ACT (scalar), 'd' = DVE (vector).  PSUM is not reachable from Pool.
ENG_A = os.environ.get("BC_ENG_A", "a")
ENG_B = os.environ.get("BC_ENG_B", "a")
ENG_C = os.environ.get("BC_ENG_C", "ad")
# bias mechanism: "pe" = 1-partition ones-matmul into PSUM, "evac" = per-i
# Identity+bias during C evacuation
CBIAS = os.environ.get("BC_CBIAS", "pe")
# intermediate dtype: bf16 (fast, known-good) or f32r (fp32 bits, accuracy fallback)
IDT_NAME = os.environ.get("BC_IDT", "bf16")

_NC_CACHE = {}
_PACK_CACHE = {}


def _dt_of(name):
    return {
        "fp32": mybir.dt.float32,
        "f32r": mybir.dt.float32r,
        "bf16": mybir.dt.bfloat16,
    }[name]


def _bf16():
    import ml_dtypes
    return ml_dtypes.bfloat16


# ---------------------------------------------------------------- sigma pack
def _sigma_components():
    """slot c, quadrant Q -> ("re"|"im", f). Pairs (2c+1, 2c+2) for c<31,
    slot 31 holds (63 complex, 0 real, 64 real)."""
    comp = {}
    for c in range(32):
        fa = 2 * c + 1 if c < 31 else 63
        comp[(0, c)] = ("re", fa)
        comp[(1, c)] = ("im", fa)
        if c < 31:
            comp[(2, c)] = ("re", 2 * c + 2)
            comp[(3, c)] = ("im", 2 * c + 2)
        else:
            comp[(2, c)] = ("re", 0)
            comp[(3, c)] = ("re", 64)
    return comp


def _pack_const():
    """Input-independent factor matrices Csig [s, m] and Esig [m, t]."""
    if "const" in _PACK_CACHE:
        return _PACK_CACHE["const"]
    comp = _sigma_components()
    s = np.arange(BS)
    Csig = np.zeros((BS, 128), dtype=np.float64)
    Esig = np.zeros((128, BS), dtype=np.float64)
    for (Q, c), (typ, f) in comp.items():
        m = 32 * Q + c
        ang = 2 * np.pi * f * s / BS
        a = (1.0 if f in (0, 64) else 2.0) / BS
        if typ == "re":
            Csig[:, m] = np.cos(ang)
            Esig[m, :] = a * np.cos(ang)
        else:
            Csig[:, m] = -np.sin(ang)
            Esig[m, :] = -a * np.sin(ang)
    out = (Csig.astype(np.float32), np.ascontiguousarray(Esig.astype(np.float32)))
    _PACK_CACHE["const"] = out
    return out


def _pack_wb(W):
    """Frequency-domain block-diagonal weights WBt [row=(Qr,j), slot, col=(Qc,i)]."""
    comp = _sigma_components()
    Wf = np.fft.fft(W.astype(np.float64), axis=-1)
    Wfr, Wfi = Wf.real, Wf.imag
    WB = np.zeros((32, 128, 128), dtype=np.float64)
    for c in range(32):
        for (qre, qim) in ((0, 1), (2, 3)):
            typ_im = comp[(qim, c)][0]
            f = comp[(qre, c)][1]
            if typ_im == "im":
                wr = Wfr[:, :, f].T  # [j, i]
                wi = Wfi[:, :, f].T
                WB[c, qre*32:(qre+1)*32, qre*32:(qre+1)*32] = wr
                WB[c, qim*32:(qim+1)*32, qre*32:(qre+1)*32] = wi
                WB[c, qre*32:(qre+1)*32, qim*32:(qim+1)*32] = -wi
                WB[c, qim*32:(qim+1)*32, qim*32:(qim+1)*32] = wr
            else:
                f2 = comp[(qim, c)][1]
                WB[c, qre*32:(qre+1)*32, qre*32:(qre+1)*32] = Wfr[:, :, f].T
                WB[c, qim*32:(qim+1)*32, qim*32:(qim+1)*32] = Wfr[:, :, f2].T
    return np.ascontiguousarray(
        WB.transpose(1, 0, 2).astype(np.float32)  # [row, slot, col]
    )


# --------------------------------------------------------------- fft3 build
def _build_fft3():
    key = ("fft3", ENG_A, ENG_B, ENG_C, CBIAS, IDT_NAME)
    if key in _NC_CACHE:
        return _NC_CACHE[key]
    DT = mybir.dt.bfloat16            # x / csig / output dtype
    IDT = _dt_of(IDT_NAME)            # intermediate + B/C weights dtype
    f32 = mybir.dt.float32

    nc = bacc.Bacc(None, target_bir_lowering=False, debug=False)

    xT = nc.dram_tensor("xT", [BS, KI, BC], DT, kind="ExternalInput")
    Csig_d = nc.dram_tensor("Csig", [BS, 128], DT, kind="ExternalInput")
    WBt_d = nc.dram_tensor("WBt", [128, 32, 128], IDT, kind="ExternalInput")
    Esig_d = nc.dram_tensor("Esig", [128, BS], IDT, kind="ExternalInput")
    bTT_d = nc.dram_tensor("bTT", [KO, BS], IDT, kind="ExternalInput")
    mask_d = nc.dram_tensor("mask", [KO, 512], IDT, kind="ExternalInput")
    # out[t, (h,k8), (p,b16), i32] -> host reshapes to [t, b, i]
    outT = nc.dram_tensor("outT", [BS, 16, 4, BH], DT, kind="ExternalOutput")
    if LDWOPT:
        nc.dram_tensor("ldwopt_tag", [1, 1], f32, kind="ExternalInput")

    def eng_iter(pattern):
        n = [0]

        def next_eng():
            e = pattern[n[0] % len(pattern)]
            n[0] += 1
            return e
        return next_eng

    ea, eb, ec = eng_iter(ENG_A), eng_iter(ENG_B), eng_iter(ENG_C)

    def evac(eng, out, in_, bias=None):
        """PSUM -> SBUF evacuation copy on ACT ('a') or DVE ('d')."""
        if bias is not None:
            if eng == "a":
                nc.scalar.activation(
                    out=out, in_=in_,
                    func=mybir.ActivationFunctionType.Identity, bias=bias,
                )
            else:
                nc.vector.tensor_scalar_add(out=out, in0=in_, scalar1=bias)
        else:
            if eng == "a":
                nc.scalar.activation(
                    out=out, in_=in_, func=mybir.ActivationFunctionType.Copy
                )
            else:
                nc.vector.tensor_copy(out=out, in_=in_)

    with TileContext(nc) as tc:
        with tc.tile_pool(name="consts", bufs=1) as cpool, \
             tc.tile_pool(name="stage", bufs=KI) as spool, \
             tc.tile_pool(name="big1", bufs=2) as big1, \
             tc.tile_pool(name="big2", bufs=2) as big2, \
             tc.tile_pool(name="o", bufs=4) as opool, \
             tc.tile_pool(name="ps", bufs=4, space="PSUM") as pspool:

            csig = cpool.tile([BS, 128], DT)
            esig = cpool.tile([128, BS], IDT)
            wb = cpool.tile([128, 32, 128], IDT)
            btt = cpool.tile([KO, BS], IDT)
            mask = cpool.tile([KO, 512], IDT)
            # csig gates the first matmuls; the rest is not needed until
            # stage B/C, so ship it on the scalar queue out of the way
            nc.sync.dma_start(out=csig, in_=Csig_d[:, :])
            nc.scalar.dma_start(out=wb, in_=WBt_d[:, :, :])
            nc.scalar.dma_start(out=esig, in_=Esig_d[:, :])
            nc.scalar.dma_start(out=btt, in_=bTT_d[:, :])
            nc.scalar.dma_start(out=mask, in_=mask_d[:, :])

            # all intermediates b-major: both stream transposes get contiguous
            # input AND output (strided-output transposes run 4x slow)
            xf = [big1.tile([128, BH, KI], IDT, tag="big1", name=f"xf{h}")
                  for h in range(NSPLIT)]
            z = [None] * NSPLIT
            yz = [None] * NSPLIT
            yw = [None] * NSPLIT

            # full-width x staging: one DMA per j, tiles live across halves;
            # loads are emitted inside stage_a(0) so the first matmuls are not
            # gated on the whole input transfer
            st = [spool.tile([BS, BC], DT, tag="stage", name=f"st{j}")
                  for j in range(KI)]

            def stage_a_full():
                # DFT: XF[m, b, j] = sum_s Csig[s, m] * xd[s, j, b]
                # full-width (512) matmuls, evac split per half for T1 pipelining
                for jp in range(KI // 2):
                    for d in range(2):
                        nc.sync.dma_start(out=st[2 * jp + d],
                                          in_=xT[:, 2 * jp + d, :])
                    ps = pspool.tile([128, 2, BC], f32, tag="ps",
                                     name=f"psa{jp}")
                    for d in range(2):
                        nc.tensor.matmul(
                            ps[:, d, :], csig, st[2 * jp + d],
                            start=True, stop=True,
                        )
                    for h in range(NSPLIT):
                        evac(ea(), xf[h][:, :, 2 * jp:2 * jp + 2],
                             ps[:, :, h * BH:(h + 1) * BH].transpose([0, 2, 1]))

            def t1(h):
                # Z[(Q,j), b, c] = XF[(Q,c), b, j]  (contig in AND out)
                z[h] = big2.tile([128, BH, 32], IDT, tag="big2", name=f"z{h}")
                nc.vector.transpose(out=z[h], in_=xf[h])

            def stage_b(h):
                # per-slot block-diagonal frequency matmul (moving stride 32)
                yz[h] = big1.tile([128, BH, 32], IDT, tag="big1", name=f"yz{h}")
                for q in range(8):
                    ps = pspool.tile([128, 4, BH], f32, tag="ps",
                                     name=f"psb{q}_{h}")
                    for d in range(4):
                        c = 4 * q + d
                        nc.tensor.matmul(
                            ps[:, d, :], wb[:, c, :], z[h][:, :, c],
                            start=True, stop=True,
                        )
                    evac(eb(), yz[h][:, :, 4 * q:4 * q + 4],
                         ps.transpose([0, 2, 1]))

            def t2(h):
                # YW[(Q,c), b, i] = YZ[(Q,i), b, c]  (contig in AND out)
                yw[h] = big2.tile([128, BH, 32], IDT, tag="big2", name=f"yw{h}")
                nc.vector.transpose(out=yw[h], in_=yz[h])

            def stage_c(h):
                # iDFT over 16-batch chunks: moving yw[:, b16, :] is contiguous,
                # esig stationary shared, bias added exactly via mask matmul.
                for q in range(8):
                    ps = pspool.tile([128, 4, BH], f32, tag="ps",
                                     name=f"psc{q}_{h}")
                    for p in range(2):
                        bsl = slice(32 * q + 16 * p, 32 * q + 16 * p + 16)
                        nc.tensor.matmul(
                            ps[:, 2 * p:2 * p + 2, :], esig, yw[h][:, bsl, :],
                            start=True, stop=False,
                        )
                        nc.tensor.matmul(
                            ps[:, 2 * p:2 * p + 2, :], btt, mask,
                            start=False, stop=True, skip_group_check=True,
                        )
                    ob = opool.tile([128, 4, BH], DT, tag="o")
                    evac(ec(), ob, ps)
                    nc.scalar.dma_start(
                        out=outT[:, 8 * h + q, :, :], in_=ob
                    )

            # dataflow-ordered emission for pipelined queues
            stage_a_full()
            t1(0)
            stage_b(0)
            t1(1)
            t2(0)
            stage_b(1)
            stage_c(0)
            t2(1)
            stage_c(1)

    nc.compile()
    _NC_CACHE[key] = nc
    return nc


def _prep_fft3(x, W, D, bias):
    bf = _bf16()
    Csig, Esig = _pack_const()
    WBt = _pack_wb(W)
    Csig16 = Csig.astype(bf)
    wdt = bf if IDT_NAME == "bf16" else np.float32
    Esig_w = np.ascontiguousarray(Esig).astype(wdt)
    WBt_w = WBt.astype(wdt)
    bTT = np.ascontiguousarray(bias.reshape(KO, BS)).astype(wdt)
    msk = np.zeros((KO, 512), dtype=np.float32)
    for n in range(512):
        msk[n % 32, n] = 1.0
    msk_w = msk.astype(wdt)
    xd = (x * D[None, :]).astype(np.float32)
    in_maps = []
    for c in range(NCORES):
        xs = xd[c * BC:(c + 1) * BC, :]
        xTc = np.ascontiguousarray(
            xs.reshape(BC, KI, BS).transpose(2, 1, 0)
        ).astype(bf)
        im = {"xT": xTc, "Csig": Csig16, "WBt": WBt_w, "Esig": Esig_w,
              "bTT": bTT, "mask": msk_w}
        if LDWOPT:
            im["ldwopt_tag"] = np.zeros((1, 1), dtype=np.float32)
        in_maps.append(im)
    return in_maps


# ---------------------------------------------------------------- fft build
def _build_fft(mm_dtype):
    key = ("fft", mm_dtype)
    if key in _NC_CACHE:
        return _NC_CACHE[key]
    DT = _dt_of(mm_dtype)
    f32 = mybir.dt.float32
    need_round = DT == mybir.dt.float32r
    nsplit = 2
    bh = BC // nsplit

    nc = bacc.Bacc(None, target_bir_lowering=False, debug=False)

    xT = nc.dram_tensor("xT", [BS, KI, BC], DT, kind="ExternalInput")
    Csig_d = nc.dram_tensor("Csig", [BS, 128], f32, kind="ExternalInput")
    WBt_d = nc.dram_tensor("WBt", [128, 32, 128], DT, kind="ExternalInput")
    Esig_d = nc.dram_tensor("Esig", [128, BS], DT, kind="ExternalInput")
    Dt_d = nc.dram_tensor("Dt", [BS, KI], f32, kind="ExternalInput")
    bT_d = nc.dram_tensor("bT", [BS, KO], f32, kind="ExternalInput")
    outT = nc.dram_tensor("outT", [KO, BS, BC], f32, kind="ExternalOutput")
    if LDWOPT:
        nc.dram_tensor("ldwopt_tag", [1, 1], f32, kind="ExternalInput")

    def do_copy(k, out, in_, act_frac=2):
        if k % act_frac == 0:
            nc.vector.tensor_copy(out=out, in_=in_)
        else:
            nc.scalar.activation(
                out=out, in_=in_, func=mybir.ActivationFunctionType.Copy
            )

    with TileContext(nc) as tc:
        with tc.tile_pool(name="consts", bufs=1) as cpool, \
             tc.tile_pool(name="stage", bufs=6) as spool, \
             tc.tile_pool(name="big1", bufs=2) as big1, \
             tc.tile_pool(name="big2", bufs=2) as big2, \
             tc.tile_pool(name="big3", bufs=1) as big3, \
             tc.tile_pool(name="o", bufs=6) as opool, \
             tc.tile_pool(name="psAll", bufs=8, space="PSUM") as psAll:

            psA = psB = psC = psAll
            csig = cpool.tile([BS, 128], f32)
            esig = cpool.tile([128, BS], DT)
            dt_t = cpool.tile([BS, KI], f32)
            bt_t = cpool.tile([BS, KO], f32)
            wb = cpool.tile([128, 32, 128], DT)
            if need_round:
                cd = big3.tile([BS, KI, 128], DT, tag="big3", name="cd")
            else:
                cd = cpool.tile([BS, KI, 128], DT)
            nc.sync.dma_start(out=csig, in_=Csig_d[:, :])
            nc.sync.dma_start(out=esig, in_=Esig_d[:, :])
            nc.sync.dma_start(out=dt_t, in_=Dt_d[:, :])
            nc.sync.dma_start(out=bt_t, in_=bT_d[:, :])
            nc.sync.dma_start(out=wb, in_=WBt_d[:, :, :])

            for j in range(KI):
                nc.vector.tensor_scalar_mul(
                    out=cd[:, j, :], in0=csig, scalar1=dt_t[:, j : j + 1]
                )

            tdt = f32 if need_round else DT

            xf = [big1.tile([128, KI, bh], tdt, tag="big1", name=f"xf{h}")
                  for h in range(nsplit)]
            for j in range(KI):
                st = spool.tile([BS, BC], DT, tag="stage")
                nc.sync.dma_start(out=st, in_=xT[:, j, :])
                ps = psA.tile([128, BC], f32, tag="ps", name=f"psa{j}")
                nc.tensor.matmul(ps, cd[:, j, :], st, start=True, stop=True)
                for h in range(nsplit):
                    do_copy(j + h, xf[h][:, j, :], ps[:, h * bh : (h + 1) * bh])

            for h in range(nsplit):
                z = big2.tile([128, 32, bh], tdt, tag="big2", name=f"z{h}")
                nc.vector.transpose(out=z.transpose([0, 2, 1]), in_=xf[h].transpose([0, 2, 1]))
                if need_round:
                    zr = big3.tile([128, 32, bh], DT, tag="big3", name=f"zr{h}")
                    nc.gpsimd.dma_start(out=zr, in_=z)
                    z = zr
                yz = big1.tile([128, 32, bh], tdt, tag="big1", name=f"yz{h}")
                for c in range(32):
                    ps = psB.tile([128, bh], f32, tag="ps", name=f"psb{c}_{h}")
                    nc.tensor.matmul(ps, wb[:, c, :], z[:, c, :], start=True, stop=True)
                    do_copy(c, yz[:, c, :], ps, act_frac=4)
                yw = big2.tile([128, 32, bh], tdt, tag="big2", name=f"yw{h}")
                nc.vector.transpose(out=yw.transpose([0, 2, 1]), in_=yz.transpose([0, 2, 1]))
                if need_round:
                    ywr = big3.tile([128, 32, bh], DT, tag="big3", name=f"ywr{h}")
                    nc.gpsimd.dma_start(out=ywr, in_=yw)
                    yw = ywr
                for i in range(0, KO, 2):
                    ps = psC.tile([128, 2, bh], f32, tag="ps", name=f"psc{i}_{h}")
                    nc.tensor.matmul(
                        ps, esig, yw[:, i : i + 2, :], start=True, stop=True
                    )
                    for d in range(2):
                        oi = opool.tile([BS, bh], f32, tag="o")
                        nc.scalar.activation(
                            out=oi, in_=ps[:, d, :],
                            func=mybir.ActivationFunctionType.Identity,
                            bias=bt_t[:, i + d : i + d + 1],
                        )
                        nc.sync.dma_start(
                            out=outT[i + d, :, h * bh : (h + 1) * bh], in_=oi
                        )

    nc.compile()
    _NC_CACHE[key] = nc
    return nc


def _prep_fft(x, W, D, bias):
    Csig, Esig = _pack_const()
    WBt = _pack_wb(W)
    Dt = np.ascontiguousarray(D.reshape(KI, BS).T)
    bT = np.ascontiguousarray(bias.reshape(KO, BS).T)
    in_maps = []
    for c in range(NCORES):
        xs = x[c * BC : (c + 1) * BC, :]
        xTc = np.ascontiguousarray(xs.reshape(BC, KI, BS).transpose(2, 1, 0))
        im = {"xT": xTc, "Csig": Csig, "WBt": WBt, "Esig": Esig, "Dt": Dt, "bT": bT}
        if LDWOPT:
            im["ldwopt_tag"] = np.zeros((1, 1), dtype=np.float32)
        in_maps.append(im)
    return in_maps


# ------------------------------------------------------------------- driver
def _run(inputs, trace=False):
    x = np.asarray(inputs["x"], dtype=np.float32)
    W = np.asarray(inputs["W"], dtype=np.float32)
    D = np.asarray(inputs["D_bernoulli"], dtype=np.float32)
    bias = np.asarray(inputs["bias"], dtype=np.float32)

    if IMPL == "fft3":
        nc = _build_fft3()
        in_maps = _prep_fft3(x, W, D, bias)
    else:
        nc = _build_fft(MM_DTYPE)
        in_maps = _prep_fft(x, W, D, bias)

    res = run_bass_kernel_spmd(nc, in_maps, list(range(NCORES)), trace=trace)
    out = np.empty((BATCH, D_OUT), dtype=np.float32)
    for c in range(NCORES):
        oT = np.asarray(res.results[c]["outT"], dtype=np.float32)
        if IMPL == "fft3":
            # [t, (h,q), (p, b16, i)] -> [b, i*128+t]
            ob = oT.reshape(BS, 16, 2, 16, KO).transpose(1, 2, 3, 4, 0)
            out[c * BC : (c + 1) * BC, :] = ob.reshape(BC, D_OUT)
        else:
            # [i, t, b]
            out[c * BC : (c + 1) * BC, :] = (
                oT.transpose(2, 0, 1).reshape(BC, D_OUT)
            )
    return out, res


def kernel(**inputs) -> np.ndarray:
    out, _ = _run(inputs, trace=False)
    return out
